# revision 5
# baseline (speedup 1.0000x reference)
"""DistSAGEConv forward on 8 Trainium2 NeuronCores (Bass/Tile), bf16 compute.

Math (matches the reference):
    h_neigh = segment_mean(local_feats[src], dst)            # [N, D]
    out     = relu(local_feats @ W_self.T + h_neigh @ W_neigh.T + b)

Distribution: nodes (and their incident dst edges) are sharded across the 8
cores, 6250 nodes each; the weights/bias are replicated; the full feature
table is replicated into every core's HBM so "remote neighbor features" are
indirect-DMA gathers from the local copy (the halo exchange of the Dist
semantics collapses to a local gather because we receive full inputs).

Per core, per dst-tile of 128 nodes:
  1. dma_gather the tile's incident src rows (bf16, 1KB each) into SBUF in
     edge order, padded to chunks of 128.  Indices are int16, so the
     50000-row table is addressed as two 25000-row halves (edges pre-sorted
     into the two halves on host).
  2. For each 128-edge chunk, build the one-hot S[e, j] = (dst_id[e] == j)
     in bf16 with one vector-engine tensor_scalar (iota == per-partition
     dst id), then accumulate psum_h[128 dst, 512] += S.T @ G_chunk on the
     tensor engine.  1/deg is applied afterwards as a per-partition scale
     during the PSUM->SBUF copy, so S stays exactly representable.
  3. Transpose h via PE matmuls against identity; accumulate
     psum_out = bias (K=1 matmul) + X_tile @ W_self.T + h @ W_neigh.T,
     then ReLU on the scalar engine and DMA the tile out in fp32.

Edge bookkeeping (which edges belong to which tile/half, degrees, padding)
is integer preprocessing done on host with numpy; all floating-point math
happens on device (inputs are cast to bf16 host-side, output is fp32).
"""

import ml_dtypes
import numpy as np

from concourse import bass, bacc, mybir, tile
from concourse.bass_utils import run_bass_kernel_spmd

F32 = mybir.dt.float32
BF16 = mybir.dt.bfloat16
I16 = mybir.dt.int16

N_NODES = 50000
N_EDGES = 800000
D = 512
NCORES = 8
NPC = N_NODES // NCORES          # 6250 nodes per core
P = 128                          # partitions / tile rows
NT = (NPC + P - 1) // P          # 49 dst tiles per core (last has 106 rows)
HALF = N_NODES // 2              # int16-addressable table half
GMAX = 6                         # >=1024 idx in one dma_gather wedges the HW


class Plan:
    """Compile-time structure shared by all 8 cores (program is SPMD)."""

    def __init__(self, n_nodes, npc, half, tiles):
        self.n_nodes = n_nodes
        self.npc = npc
        self.half = half
        # tiles: list of (rows, cA, cB) -- cA/cB = 128-edge chunks for the
        # low/high table half, maxed across cores so one program fits all.
        self.tiles = tiles
        self.idx_off = []
        self.meta_off = []
        io = mo = 0
        for _, ca, cb in tiles:
            self.idx_off.append(io)
            self.meta_off.append(mo)
            io += (ca + cb) * 8          # int16 idx columns (16-wrap)
            mo += ca + cb                # one meta column per chunk
        self.sum_idx = io
        self.sum_ch = mo
        self.ch_max = max(ca + cb for _, ca, cb in tiles)

    def key(self):
        return (self.n_nodes, self.npc, self.half, tuple(self.tiles))


def _prepare(local_feats, src, dst, W_self, W_neigh, b,
             n_nodes=N_NODES, ncores=NCORES):
    """Host-side integer preprocessing -> (plan, in_maps)."""
    npc = n_nodes // ncores
    nt = (npc + P - 1) // P
    half = n_nodes // 2
    feats = np.asarray(local_feats, dtype=np.float32)
    feats_bf = np.ascontiguousarray(feats.astype(ml_dtypes.bfloat16))
    src = np.asarray(src).astype(np.int64)
    dst = np.asarray(dst).astype(np.int64)

    deg = np.bincount(dst, minlength=n_nodes).astype(np.float32)
    inv_node = (1.0 / np.maximum(deg, 1.0)).astype(np.float32)

    core_of = dst // npc
    local = dst - core_of * npc
    t_of = local // P
    r_of = (local % P).astype(np.float32)
    hi = (src >= half).astype(np.int64)
    key = (core_of * nt + t_of) * 2 + hi
    order = np.argsort(key, kind="stable")
    skey = key[order]
    ssrc = src[order]
    srid = r_of[order]
    # segment boundaries for each (core, tile, half)
    bounds = np.searchsorted(skey, np.arange(ncores * nt * 2 + 1))

    def seg(c, t, h):
        k = (c * nt + t) * 2 + h
        return bounds[k], bounds[k + 1]

    # per-(t) chunk counts, maxed across cores
    tiles = []
    for t in range(nt):
        rows = min(P, npc - t * P)
        na = max(seg(c, t, 0)[1] - seg(c, t, 0)[0] for c in range(ncores))
        nb = max(seg(c, t, 1)[1] - seg(c, t, 1)[0] for c in range(ncores))
        ca = (na + P - 1) // P
        cb = (nb + P - 1) // P
        tiles.append((rows, ca, cb))
    plan = Plan(n_nodes, npc, half, tiles)

    # replicated constants (bf16)
    wts = np.ascontiguousarray(
        W_self.T.astype(ml_dtypes.bfloat16).reshape(4, P, D).transpose(1, 0, 2))
    wtn = np.ascontiguousarray(
        W_neigh.T.astype(ml_dtypes.bfloat16).reshape(4, P, D).transpose(1, 0, 2))
    bias = np.ascontiguousarray(b.astype(ml_dtypes.bfloat16).reshape(1, D))
    ones = np.ones((1, P), dtype=ml_dtypes.bfloat16)
    ident = np.eye(P, dtype=ml_dtypes.bfloat16)
    iota = np.ascontiguousarray(
        np.tile(np.arange(P, dtype=ml_dtypes.bfloat16), (P, 1)))

    in_maps = []
    for c in range(ncores):
        idx_cols = []
        rid_cols = []
        for t in range(nt):
            rows, ca, cb = plan.tiles[t]
            for h, cn in ((0, ca), (1, cb)):
                lo, hiq = seg(c, t, h)
                n = hiq - lo
                npad = cn * P
                iv = np.zeros(npad, dtype=np.int16)
                iv[:n] = (ssrc[lo:hiq] - h * half).astype(np.int16)
                # idx wrap: i -> [i%16, i//16], replicated to 128 partitions
                m = iv.reshape(npad // 16, 16).T
                idx_cols.append(np.tile(m, (8, 1)))
                rv = np.full(npad, 255.0, dtype=np.float32)
                rv[:n] = srid[lo:hiq]
                # meta wrap: i -> [i%128, i//128]
                rid_cols.append(rv.reshape(cn, P).T)
        eidx = np.ascontiguousarray(np.concatenate(idx_cols, axis=1))
        edst = np.ascontiguousarray(
            np.concatenate(rid_cols, axis=1).astype(np.float32))

        # per-node 1/deg for this core's dst rows: [P, nt] (partition = row)
        invc = np.zeros((nt * P,), dtype=np.float32)
        invc[:npc] = inv_node[c * npc:(c + 1) * npc]
        einv = np.ascontiguousarray(invc.reshape(nt, P).T)

        # self-chunk, transposed + tiled: xt[t, p, f, j] = Xc[t*128+j, f*128+p]
        xc = np.zeros((nt * P, D), dtype=ml_dtypes.bfloat16)
        xc[:npc] = feats_bf[c * npc:(c + 1) * npc]
        xt = np.ascontiguousarray(
            xc.reshape(nt, P, 4, P).transpose(0, 3, 2, 1))

        in_maps.append({
            "feats": feats_bf,
            "xt": xt,
            "wts": wts,
            "wtn": wtn,
            "bias": bias,
            "ones": ones,
            "ident": ident,
            "iota": iota,
            "eidx": eidx,
            "edst": edst,
            "einv": einv,
        })
    return plan, in_maps


def build(plan):
    """Build + compile the SPMD Bass program for one core."""
    nc = bacc.Bacc("TRN2", target_bir_lowering=False, debug=False,
                   enable_asserts=False, num_devices=NCORES)
    n_nodes, npc, half = plan.n_nodes, plan.npc, plan.half
    nt = len(plan.tiles)

    feats = nc.dram_tensor("feats", [n_nodes, D], BF16, kind="ExternalInput")
    xt = nc.dram_tensor("xt", [nt, P, 4, P], BF16, kind="ExternalInput")
    wts = nc.dram_tensor("wts", [P, 4, D], BF16, kind="ExternalInput")
    wtn = nc.dram_tensor("wtn", [P, 4, D], BF16, kind="ExternalInput")
    bias = nc.dram_tensor("bias", [1, D], BF16, kind="ExternalInput")
    ones = nc.dram_tensor("ones", [1, P], BF16, kind="ExternalInput")
    ident = nc.dram_tensor("ident", [P, P], BF16, kind="ExternalInput")
    iota = nc.dram_tensor("iota", [P, P], BF16, kind="ExternalInput")
    eidx = nc.dram_tensor("eidx", [P, plan.sum_idx], I16, kind="ExternalInput")
    edst = nc.dram_tensor("edst", [P, plan.sum_ch], F32, kind="ExternalInput")
    einv = nc.dram_tensor("einv", [P, nt], F32, kind="ExternalInput")
    out = nc.dram_tensor("out", [npc, D], F32, kind="ExternalOutput")

    AF = mybir.ActivationFunctionType
    OP = mybir.AluOpType

    with tile.TileContext(nc) as tc:
        with (
            tc.tile_pool(name="const", bufs=1) as cpool,
            tc.tile_pool(name="g", bufs=3) as gpool,
            tc.tile_pool(name="s", bufs=4) as spool,
            tc.tile_pool(name="x", bufs=2) as xpool,
            tc.tile_pool(name="h", bufs=2) as hpool,
            tc.tile_pool(name="ht", bufs=2) as htpool,
            tc.tile_pool(name="o", bufs=2) as opool,
            tc.tile_pool(name="ph", bufs=2, space="PSUM") as phpool,
            tc.tile_pool(name="ptr", bufs=2, space="PSUM") as ptrpool,
            tc.tile_pool(name="po", bufs=2, space="PSUM") as popool,
        ):
            # resident constants
            wts_s = cpool.tile([P, 4, D], BF16, tag="wts")
            nc.sync.dma_start(wts_s[:], wts[:])
            wtn_s = cpool.tile([P, 4, D], BF16, tag="wtn")
            nc.sync.dma_start(wtn_s[:], wtn[:])
            bias_s = cpool.tile([1, D], BF16, tag="bias")
            nc.sync.dma_start(bias_s[:], bias[:])
            ones_s = cpool.tile([1, P], BF16, tag="ones")
            nc.sync.dma_start(ones_s[:], ones[:])
            ident_s = cpool.tile([P, P], BF16, tag="ident")
            nc.sync.dma_start(ident_s[:], ident[:])
            iota_s = cpool.tile([P, P], BF16, tag="iota")
            nc.sync.dma_start(iota_s[:], iota[:])
            idx_s = cpool.tile([P, plan.sum_idx], I16, tag="eidx")
            nc.sync.dma_start(idx_s[:], eidx[:])
            dst_s = cpool.tile([P, plan.sum_ch], F32, tag="edst")
            nc.sync.dma_start(dst_s[:], edst[:])
            inv_s = cpool.tile([P, nt], F32, tag="einv")
            nc.sync.dma_start(inv_s[:], einv[:])

            feats_a = feats[0:half, :]
            feats_b = feats[half:n_nodes, :]

            for t in range(nt):
                rows, ca, cb = plan.tiles[t]
                ch = ca + cb
                io = plan.idx_off[t]
                mo = plan.meta_off[t]

                g = gpool.tile([P, plan.ch_max, D], BF16, tag="g")
                for base, cn, src_ap in ((0, ca, feats_a),
                                         (ca, cb, feats_b)):
                    for c0 in range(0, cn, GMAX):
                        cw = min(GMAX, cn - c0)
                        nc.gpsimd.dma_gather(
                            g[:, base + c0:base + c0 + cw, :], src_ap,
                            idx_s[:, io + (base + c0) * 8:
                                  io + (base + c0 + cw) * 8],
                            cw * P, cw * P, D)

                xt_t = xpool.tile([P, 4, P], BF16, tag="x")
                nc.sync.dma_start(xt_t[:], xt[t])

                # aggregation: psum_h[dst, feat] += S_c.T @ G_c
                ph = phpool.tile([P, D], F32, tag="ph")
                for c in range(ch):
                    s = spool.tile([P, P], BF16, tag="s")
                    nc.vector.tensor_scalar(
                        s[:], iota_s[:],
                        dst_s[:, mo + c:mo + c + 1], None,
                        op0=OP.is_equal)
                    nc.tensor.matmul(
                        ph[:], s[:], g[:, c, :],
                        start=(c == 0), stop=(c == ch - 1))

                # h = ph * (1/deg)  (per-partition scale during PSUM->SBUF)
                h = hpool.tile([P, D], BF16, tag="h")
                nc.vector.tensor_scalar(
                    h[:], ph[:], inv_s[:, t:t + 1], None, op0=OP.mult)

                # transpose h -> hT as plain matmuls against identity
                ht = htpool.tile([P, 4, P], BF16, tag="ht")
                ptr = ptrpool.tile([P, 4, P], F32, tag="ptr")
                for f in range(4):
                    nc.tensor.matmul(
                        ptr[:, f, :], h[:, f * P:(f + 1) * P], ident_s[:],
                        start=True, stop=True)
                nc.vector.tensor_copy(ht[:], ptr[:])

                # out = relu(bias + X @ Wself.T + h @ Wneigh.T)
                po = popool.tile([P, D], F32, tag="po")
                nc.tensor.matmul(po[:], ones_s[:], bias_s[:],
                                 start=True, stop=False)
                for f in range(4):
                    nc.tensor.matmul(po[:], xt_t[:, f, :], wts_s[:, f, :],
                                     start=False, stop=False)
                    nc.tensor.matmul(po[:], ht[:, f, :], wtn_s[:, f, :],
                                     start=False, stop=(f == 3))

                o = opool.tile([P, D], F32, tag="o")
                nc.scalar.activation(o[:], po[:], AF.Relu)
                nc.sync.dma_start(out[t * P:t * P + rows, :], o[:rows, :])

    nc.compile()
    return nc


_cache = {}


def _get_nc(plan):
    k = plan.key()
    if k not in _cache:
        _cache[k] = build(plan)
    return _cache[k]


def kernel(local_feats, src, dst, layer=None, W_self=None, W_neigh=None,
           b=None, **_unused):
    plan, in_maps = _prepare(local_feats, src, dst, W_self, W_neigh, b)
    nc = _get_nc(plan)
    res = run_bass_kernel_spmd(nc, in_maps, core_ids=list(range(NCORES)))
    return np.concatenate([res.results[c]["out"] for c in range(NCORES)],
                          axis=0)


# revision 7
# speedup vs baseline: 1.2150x; 1.2150x over previous
"""DistSAGEConv forward on 8 Trainium2 NeuronCores (Bass/Tile), bf16 compute.

Math (matches the reference):
    h_neigh = segment_mean(local_feats[src], dst)            # [N, D]
    out     = relu(local_feats @ W_self.T + h_neigh @ W_neigh.T + b)

Distribution: nodes (and their incident dst edges) are sharded across the 8
cores, 6250 nodes each; the weights/bias are replicated; the full feature
table is replicated into every core's HBM so "remote neighbor features" are
indirect-DMA gathers from the local copy (the halo exchange of the Dist
semantics collapses to a local gather because we receive full inputs).

The kernel is bound by SWDGE descriptor generation on the GpSimd engine
(~9 ns per gathered row), so the design minimizes gathered rows and keeps
every other engine strictly below that wall:

  1. Sources are deduplicated per dst-tile and the int16-indexed table is
     addressed as two OVERLAPPING halves A=[0,32768) / B=[17232,50000);
     edges in the overlap are assigned host-side to equalize chunk counts
     across cores and minimize 128-padding.
  2. The scatter matrices S[slot, dst] (multiplicity counts, bf16-exact)
     are precomputed on host and DMAed in via HWDGE -- the vector engine
     does no work at all (DVE contends with GpSimd for an SBUF port, so
     any DVE op stalls behind the descriptor stream).
  3. psum_h[128 dst, 512] += S_c.T @ G_c per 128-slot chunk on the tensor
     engine; 1/deg is applied as a per-partition scale during the
     PSUM->SBUF copy on the *scalar* engine (ACT), as are the hT copies.
  4. hT via PE transposes; out = relu(bias + X @ Wself.T + h @ Wneigh.T).

Edge bookkeeping (tile/half assignment, dedup, S construction, degrees,
padding) is integer preprocessing done on host with numpy; all
floating-point math happens on device (inputs cast to bf16, output fp32).
"""

import ml_dtypes
import numpy as np

from concourse import bass, bacc, mybir, tile
from concourse.bass_utils import run_bass_kernel_spmd

F32 = mybir.dt.float32
BF16 = mybir.dt.bfloat16
I16 = mybir.dt.int16

N_NODES = 50000
N_EDGES = 800000
D = 512
NCORES = 8
NPC = N_NODES // NCORES          # 6250 nodes per core
P = 128                          # partitions / tile rows
NT = (NPC + P - 1) // P          # 49 dst tiles per core (last has 106 rows)
A_END = 32768                    # half A covers [0, A_END)
B_OFF = N_NODES - 32768          # half B covers [B_OFF, N_NODES)
GMAX = 6                         # >=1024 idx in one dma_gather wedges the HW


class Plan:
    """Compile-time structure shared by all 8 cores (program is SPMD)."""

    def __init__(self, n_nodes, npc, tiles):
        self.n_nodes = n_nodes
        self.npc = npc
        # tiles: list of (rows, cA, cB) -- cA/cB = 128-slot chunks for the
        # low/high table half, shared across cores so one program fits all.
        self.tiles = tiles
        self.idx_off = []
        self.meta_off = []
        io = mo = 0
        for _, ca, cb in tiles:
            self.idx_off.append(io)
            self.meta_off.append(mo)
            io += (ca + cb) * 8          # int16 idx columns (16-wrap)
            mo += ca + cb                # one S chunk per slot chunk
        self.sum_idx = io
        self.sum_ch = mo
        self.ch_max = max(ca + cb for _, ca, cb in tiles)

    def key(self):
        return (self.n_nodes, self.npc, tuple(self.tiles))


def _prepare(local_feats, src, dst, W_self, W_neigh, b,
             n_nodes=N_NODES, ncores=NCORES):
    """Host-side integer preprocessing -> (plan, in_maps)."""
    npc = n_nodes // ncores
    nt = (npc + P - 1) // P
    feats_bf = np.ascontiguousarray(
        np.asarray(local_feats, dtype=np.float32).astype(ml_dtypes.bfloat16))
    src = np.asarray(src).astype(np.int64)
    dst = np.asarray(dst).astype(np.int64)

    deg = np.bincount(dst, minlength=n_nodes).astype(np.float32)
    inv_node = (1.0 / np.maximum(deg, 1.0)).astype(np.float32)

    # group edges by (core, tile); within a tile dedup sources and count
    # multiplicity per (source, dst-row)
    core_of = dst // npc
    local = dst - core_of * npc
    t_of = local // P
    r_of = local % P
    gkey = core_of * nt + t_of
    order = np.argsort(gkey, kind="stable")
    g_src = src[order]
    g_rid = r_of[order]
    bounds = np.searchsorted(gkey[order], np.arange(ncores * nt + 1))

    # pass 1: per (core, tile) unique sources split into forced-A / forced-B
    # / flexible; derive shared chunk counts (ca, cb) per tile.
    uniq = {}
    nAf = np.zeros((ncores, nt), dtype=np.int64)
    nBf = np.zeros((ncores, nt), dtype=np.int64)
    nFl = np.zeros((ncores, nt), dtype=np.int64)
    for c in range(ncores):
        for t in range(nt):
            k = c * nt + t
            u = np.unique(g_src[bounds[k]:bounds[k + 1]])
            uniq[c, t] = u
            nAf[c, t] = np.count_nonzero(u < B_OFF)
            nBf[c, t] = np.count_nonzero(u >= A_END)
            nFl[c, t] = len(u) - nAf[c, t] - nBf[c, t]
    tot = nAf + nBf + nFl

    tiles = []
    for t in range(nt):
        rows = min(P, npc - t * P)
        need = int(np.max(np.ceil(tot[:, t] / P)))
        ca_lo = int(np.max(np.ceil(nAf[:, t] / P)))
        cb_lo = int(np.max(np.ceil(nBf[:, t] / P)))
        while True:
            ok = None
            for ca in range(ca_lo, need - cb_lo + 1):
                cb = need - ca
                if np.all(nAf[:, t] + nFl[:, t] >= tot[:, t] - cb * P):
                    ok = (ca, cb)
                    break
            if ok is not None:
                break
            need += 1
        tiles.append((rows, ok[0], ok[1]))
    plan = Plan(n_nodes, npc, tiles)

    # replicated constants (bf16)
    wts = np.ascontiguousarray(
        W_self.T.astype(ml_dtypes.bfloat16).reshape(4, P, D).transpose(1, 0, 2))
    wtn = np.ascontiguousarray(
        W_neigh.T.astype(ml_dtypes.bfloat16).reshape(4, P, D).transpose(1, 0, 2))
    bias = np.ascontiguousarray(b.astype(ml_dtypes.bfloat16).reshape(1, D))
    ones = np.ones((1, P), dtype=ml_dtypes.bfloat16)
    ident = np.eye(P, dtype=ml_dtypes.bfloat16)

    in_maps = []
    for c in range(ncores):
        eidx = np.zeros((P, plan.sum_idx), dtype=np.int16)
        sdat = np.zeros((P, plan.sum_ch, P), dtype=ml_dtypes.bfloat16)
        for t in range(nt):
            rows, ca, cb = plan.tiles[t]
            k = c * nt + t
            seg_src = g_src[bounds[k]:bounds[k + 1]]
            seg_rid = g_rid[bounds[k]:bounds[k + 1]]
            u = uniq[c, t]
            # assign flexible uniques to half A up to quota
            quotaA = min(ca * P, int(nAf[c, t] + nFl[c, t]))
            nA = max(int(nAf[c, t]), int(tot[c, t]) - cb * P)
            nA = min(quotaA, max(nA, int(nAf[c, t])))
            isA = np.zeros(len(u), dtype=bool)
            isA[u < B_OFF] = True
            flex_pos = np.nonzero((u >= B_OFF) & (u < A_END))[0]
            takeA = nA - int(nAf[c, t])
            if takeA > 0:
                isA[flex_pos[:takeA]] = True
            # slot for each unique source: A -> [0, nA), B -> [ca*P, ...)
            slot = np.zeros(len(u), dtype=np.int64)
            slot[isA] = np.arange(nA)
            slot[~isA] = ca * P + np.arange(len(u) - nA)
            # idx values (relative to half base), padded to chunks of 128
            io = plan.idx_off[t]
            iv = np.zeros(((ca + cb) * P,), dtype=np.int16)
            iv[slot[isA]] = u[isA].astype(np.int16)
            iv[slot[~isA]] = (u[~isA] - B_OFF).astype(np.int16)
            m = iv.reshape(-1, 16).T        # 16-wrap, replicate to 128
            eidx[:, io:io + (ca + cb) * 8] = np.tile(m, (8, 1))
            # scatter-matrix: S[slot % 128, chunk, dst_row] += 1
            es = slot[np.searchsorted(u, seg_src)]
            mo = plan.meta_off[t]
            st32 = np.zeros((P, ca + cb, P), dtype=np.float32)
            np.add.at(st32, (es % P, es // P, seg_rid), 1.0)
            sdat[:, mo:mo + ca + cb, :] = st32.astype(ml_dtypes.bfloat16)
        # per-node 1/deg for this core's dst rows: [P, nt]
        invc = np.zeros((nt * P,), dtype=np.float32)
        invc[:npc] = inv_node[c * npc:(c + 1) * npc]
        einv = np.ascontiguousarray(invc.reshape(nt, P).T)

        # self-chunk, transposed + tiled: xt[t, p, f, j] = Xc[t*128+j, f*128+p]
        xc = np.zeros((nt * P, D), dtype=ml_dtypes.bfloat16)
        xc[:npc] = feats_bf[c * npc:(c + 1) * npc]
        xt = np.ascontiguousarray(
            xc.reshape(nt, P, 4, P).transpose(0, 3, 2, 1))

        in_maps.append({
            "feats": feats_bf,
            "xt": xt,
            "wts": wts,
            "wtn": wtn,
            "bias": bias,
            "ones": ones,
            "ident": ident,
            "eidx": np.ascontiguousarray(eidx),
            "sdat": np.ascontiguousarray(sdat),
            "einv": einv,
        })
    return plan, in_maps


def build(plan):
    """Build + compile the SPMD Bass program for one core."""
    nc = bacc.Bacc("TRN2", target_bir_lowering=False, debug=False,
                   enable_asserts=False, num_devices=NCORES)
    n_nodes, npc = plan.n_nodes, plan.npc
    nt = len(plan.tiles)

    feats = nc.dram_tensor("feats", [n_nodes, D], BF16, kind="ExternalInput")
    xt = nc.dram_tensor("xt", [nt, P, 4, P], BF16, kind="ExternalInput")
    wts = nc.dram_tensor("wts", [P, 4, D], BF16, kind="ExternalInput")
    wtn = nc.dram_tensor("wtn", [P, 4, D], BF16, kind="ExternalInput")
    bias = nc.dram_tensor("bias", [1, D], BF16, kind="ExternalInput")
    ones = nc.dram_tensor("ones", [1, P], BF16, kind="ExternalInput")
    ident = nc.dram_tensor("ident", [P, P], BF16, kind="ExternalInput")
    eidx = nc.dram_tensor("eidx", [P, plan.sum_idx], I16, kind="ExternalInput")
    sdat = nc.dram_tensor("sdat", [P, plan.sum_ch, P], BF16,
                          kind="ExternalInput")
    einv = nc.dram_tensor("einv", [P, nt], F32, kind="ExternalInput")
    out = nc.dram_tensor("out", [npc, D], F32, kind="ExternalOutput")

    AF = mybir.ActivationFunctionType

    with tile.TileContext(nc) as tc:
        with (
            tc.tile_pool(name="const", bufs=1) as cpool,
            tc.tile_pool(name="g", bufs=3) as gpool,
            tc.tile_pool(name="s", bufs=3) as spool,
            tc.tile_pool(name="x", bufs=2) as xpool,
            tc.tile_pool(name="h", bufs=2) as hpool,
            tc.tile_pool(name="ht", bufs=2) as htpool,
            tc.tile_pool(name="o", bufs=2) as opool,
            tc.tile_pool(name="ph", bufs=2, space="PSUM") as phpool,
            tc.tile_pool(name="ptr", bufs=2, space="PSUM") as ptrpool,
            tc.tile_pool(name="po", bufs=2, space="PSUM") as popool,
        ):
            # resident constants
            wts_s = cpool.tile([P, 4, D], BF16, tag="wts")
            nc.sync.dma_start(wts_s[:], wts[:])
            wtn_s = cpool.tile([P, 4, D], BF16, tag="wtn")
            nc.sync.dma_start(wtn_s[:], wtn[:])
            bias_s = cpool.tile([1, D], BF16, tag="bias")
            nc.sync.dma_start(bias_s[:], bias[:])
            ones_s = cpool.tile([1, P], BF16, tag="ones")
            nc.sync.dma_start(ones_s[:], ones[:])
            ident_s = cpool.tile([P, P], BF16, tag="ident")
            nc.sync.dma_start(ident_s[:], ident[:])
            idx_s = cpool.tile([P, plan.sum_idx], I16, tag="eidx")
            nc.sync.dma_start(idx_s[:], eidx[:])
            inv_s = cpool.tile([P, nt], F32, tag="einv")
            nc.sync.dma_start(inv_s[:], einv[:])

            feats_a = feats[0:A_END, :]
            feats_b = feats[B_OFF:n_nodes, :]

            for t in range(nt):
                rows, ca, cb = plan.tiles[t]
                ch = ca + cb
                io = plan.idx_off[t]
                mo = plan.meta_off[t]

                g = gpool.tile([P, plan.ch_max, D], BF16, tag="g")
                for base, cn, src_ap in ((0, ca, feats_a),
                                         (ca, cb, feats_b)):
                    for c0 in range(0, cn, GMAX):
                        cw = min(GMAX, cn - c0)
                        nc.gpsimd.dma_gather(
                            g[:, base + c0:base + c0 + cw, :], src_ap,
                            idx_s[:, io + (base + c0) * 8:
                                  io + (base + c0 + cw) * 8],
                            cw * P, cw * P, D)

                s = spool.tile([P, plan.ch_max, P], BF16, tag="s")
                nc.sync.dma_start(s[:, 0:ch, :], sdat[:, mo:mo + ch, :])

                xt_t = xpool.tile([P, 4, P], BF16, tag="x")
                nc.sync.dma_start(xt_t[:], xt[t])

                # aggregation: psum_h[dst, feat] += S_c.T @ G_c
                ph = phpool.tile([P, D], F32, tag="ph")
                for c in range(ch):
                    nc.tensor.matmul(
                        ph[:], s[:, c, :], g[:, c, :],
                        start=(c == 0), stop=(c == ch - 1))

                # h = ph * (1/deg): per-partition scale during PSUM->SBUF,
                # on the scalar engine (DVE would stall behind GpSimd).
                h = hpool.tile([P, D], BF16, tag="h")
                nc.scalar.activation(h[:], ph[:], AF.Copy,
                                     scale=inv_s[:, t:t + 1])

                # transpose h -> hT as plain matmuls against identity
                ht = htpool.tile([P, 4, P], BF16, tag="ht")
                ptr = ptrpool.tile([P, 4, P], F32, tag="ptr")
                for f in range(4):
                    nc.tensor.matmul(
                        ptr[:, f, :], h[:, f * P:(f + 1) * P], ident_s[:],
                        start=True, stop=True)
                nc.scalar.activation(ht[:], ptr[:], AF.Copy)

                # out = relu(bias + X @ Wself.T + h @ Wneigh.T)
                po = popool.tile([P, D], F32, tag="po")
                nc.tensor.matmul(po[:], ones_s[:], bias_s[:],
                                 start=True, stop=False)
                for f in range(4):
                    nc.tensor.matmul(po[:], xt_t[:, f, :], wts_s[:, f, :],
                                     start=False, stop=False)
                    nc.tensor.matmul(po[:], ht[:, f, :], wtn_s[:, f, :],
                                     start=False, stop=(f == 3))

                o = opool.tile([P, D], F32, tag="o")
                nc.scalar.activation(o[:], po[:], AF.Relu)
                nc.sync.dma_start(out[t * P:t * P + rows, :], o[:rows, :])

    nc.compile()
    return nc


_cache = {}


def _get_nc(plan):
    k = plan.key()
    if k not in _cache:
        _cache[k] = build(plan)
    return _cache[k]


def kernel(local_feats, src, dst, layer=None, W_self=None, W_neigh=None,
           b=None, **_unused):
    plan, in_maps = _prepare(local_feats, src, dst, W_self, W_neigh, b)
    nc = _get_nc(plan)
    res = run_bass_kernel_spmd(nc, in_maps, core_ids=list(range(NCORES)))
    return np.concatenate([res.results[c]["out"] for c in range(NCORES)],
                          axis=0)


# revision 10
# speedup vs baseline: 1.2914x; 1.0629x over previous
"""DistSAGEConv forward on 8 Trainium2 NeuronCores (Bass/Tile), bf16 compute.

Math (matches the reference):
    h_neigh = segment_mean(local_feats[src], dst)            # [N, D]
    out     = relu(local_feats @ W_self.T + h_neigh @ W_neigh.T + b)

Distribution: nodes (and their incident dst edges) are sharded across the 8
cores, 6250 nodes each; the weights/bias are replicated; the full feature
table is replicated into every core's HBM so "remote neighbor features" are
indirect-DMA gathers from the local copy (the halo exchange of the Dist
semantics collapses to a local gather because we receive full inputs).

The kernel is bound by SWDGE descriptor generation on the GpSimd engine
(~9 ns per gathered row), so the design minimizes gathered rows and keeps
every other engine strictly below that wall:

  1. Sources are deduplicated per dst-tile and the int16-indexed table is
     addressed as two OVERLAPPING halves A=[0,32768) / B=[17232,50000);
     edges in the overlap are assigned host-side to equalize chunk counts
     across cores and minimize 128-padding.
  2. The scatter matrices S[slot, dst] (multiplicity counts, bf16-exact)
     are precomputed on host and DMAed in via HWDGE -- the vector engine
     does no work at all (DVE contends with GpSimd for an SBUF port, so
     any DVE op stalls behind the descriptor stream).
  3. psum_h[128 dst, 512] += S_c.T @ G_c per 128-slot chunk on the tensor
     engine; 1/deg is applied as a per-partition scale during the
     PSUM->SBUF copy on the *scalar* engine (ACT), as are the hT copies.
  4. hT via PE transposes; out = relu(bias + X @ Wself.T + h @ Wneigh.T).

Edge bookkeeping (tile/half assignment, dedup, S construction, degrees,
padding) is integer preprocessing done on host with numpy; all
floating-point math happens on device (inputs cast to bf16, output fp32).
"""

import ml_dtypes
import numpy as np

from concourse import bass, bacc, mybir, tile
from concourse.bass_utils import run_bass_kernel_spmd

F32 = mybir.dt.float32
BF16 = mybir.dt.bfloat16
I16 = mybir.dt.int16

N_NODES = 50000
N_EDGES = 800000
D = 512
NCORES = 8
NPC = N_NODES // NCORES          # 6250 nodes per core
P = 128                          # partitions / tile rows
NT = (NPC + P - 1) // P          # 49 dst tiles per core (last has 106 rows)
A_END = 32768                    # half A covers [0, A_END)
B_OFF = N_NODES - 32768          # half B covers [B_OFF, N_NODES)
GMAX = 20                        # chunks per dma_gather call (2560 idx ok
                                 # with single_packet=False; single-packet
                                 # calls wedge at >=1024 idx = 64 desc/eng)
NTG = 2                          # dst tiles gathered per call group


class Plan:
    """Compile-time structure shared by all 8 cores (program is SPMD).

    Tiles are gathered in groups of NTG: one gather call per table half
    per group (descriptor generation on the Q7s is the kernel's wall, so
    fewer/bigger calls win).  Within a group's G buffer the chunk columns
    are laid out [A(t0) A(t1) .. B(t0) B(t1) ..]; gcols maps each tile's
    local chunks to its G columns.  sdat stays per-tile contiguous.
    """

    def __init__(self, n_nodes, npc, tiles):
        self.n_nodes = n_nodes
        self.npc = npc
        self.tiles = tiles           # per tile: (rows, cA, cB)
        nt = len(tiles)
        self.meta_off = []
        mo = 0
        for _, ca, cb in tiles:
            self.meta_off.append(mo)
            mo += ca + cb
        self.sum_ch = mo
        self.ch_max = max(ca + cb for _, ca, cb in tiles)

        self.groups = []             # (tlist, idx_off, ca_tot, cb_tot)
        self.gcols = {}              # tile -> list of G columns (len ca+cb)
        self.gch_max = 0
        io = 0
        for g0 in range(0, nt, NTG):
            tlist = list(range(g0, min(g0 + NTG, nt)))
            ca_tot = sum(tiles[t][1] for t in tlist)
            cb_tot = sum(tiles[t][2] for t in tlist)
            aoff = 0
            boff = ca_tot
            for t in tlist:
                _, ca, cb = tiles[t]
                self.gcols[t] = (list(range(aoff, aoff + ca))
                                 + list(range(boff, boff + cb)))
                aoff += ca
                boff += cb
            self.groups.append((tlist, io, ca_tot, cb_tot))
            io += (ca_tot + cb_tot) * 8      # int16 idx columns (16-wrap)
            self.gch_max = max(self.gch_max, ca_tot + cb_tot)
        self.sum_idx = io

    def key(self):
        return (self.n_nodes, self.npc, tuple(self.tiles))


def _prepare(local_feats, src, dst, W_self, W_neigh, b,
             n_nodes=N_NODES, ncores=NCORES):
    """Host-side integer preprocessing -> (plan, in_maps)."""
    npc = n_nodes // ncores
    nt = (npc + P - 1) // P
    feats_bf = np.ascontiguousarray(
        np.asarray(local_feats, dtype=np.float32).astype(ml_dtypes.bfloat16))
    src = np.asarray(src).astype(np.int64)
    dst = np.asarray(dst).astype(np.int64)

    deg = np.bincount(dst, minlength=n_nodes).astype(np.float32)
    inv_node = (1.0 / np.maximum(deg, 1.0)).astype(np.float32)

    # group edges by (core, tile); within a tile dedup sources and count
    # multiplicity per (source, dst-row)
    core_of = dst // npc
    local = dst - core_of * npc
    t_of = local // P
    r_of = local % P
    gkey = core_of * nt + t_of
    order = np.argsort(gkey, kind="stable")
    g_src = src[order]
    g_rid = r_of[order]
    bounds = np.searchsorted(gkey[order], np.arange(ncores * nt + 1))

    # pass 1: per (core, tile) unique sources split into forced-A / forced-B
    # / flexible; derive shared chunk counts (ca, cb) per tile.
    uniq = {}
    nAf = np.zeros((ncores, nt), dtype=np.int64)
    nBf = np.zeros((ncores, nt), dtype=np.int64)
    nFl = np.zeros((ncores, nt), dtype=np.int64)
    for c in range(ncores):
        for t in range(nt):
            k = c * nt + t
            u = np.unique(g_src[bounds[k]:bounds[k + 1]])
            uniq[c, t] = u
            nAf[c, t] = np.count_nonzero(u < B_OFF)
            nBf[c, t] = np.count_nonzero(u >= A_END)
            nFl[c, t] = len(u) - nAf[c, t] - nBf[c, t]
    tot = nAf + nBf + nFl

    tiles = []
    for t in range(nt):
        rows = min(P, npc - t * P)
        need = int(np.max(np.ceil(tot[:, t] / P)))
        ca_lo = int(np.max(np.ceil(nAf[:, t] / P)))
        cb_lo = int(np.max(np.ceil(nBf[:, t] / P)))
        while True:
            ok = None
            for ca in range(ca_lo, need - cb_lo + 1):
                cb = need - ca
                if np.all(nAf[:, t] + nFl[:, t] >= tot[:, t] - cb * P):
                    ok = (ca, cb)
                    break
            if ok is not None:
                break
            need += 1
        tiles.append((rows, ok[0], ok[1]))
    plan = Plan(n_nodes, npc, tiles)

    # replicated constants (bf16)
    wts = np.ascontiguousarray(
        W_self.T.astype(ml_dtypes.bfloat16).reshape(4, P, D).transpose(1, 0, 2))
    wtn = np.ascontiguousarray(
        W_neigh.T.astype(ml_dtypes.bfloat16).reshape(4, P, D).transpose(1, 0, 2))
    bias = np.ascontiguousarray(b.astype(ml_dtypes.bfloat16).reshape(1, D))
    ones = np.ones((1, P), dtype=ml_dtypes.bfloat16)
    ident = np.eye(P, dtype=ml_dtypes.bfloat16)

    in_maps = []
    for c in range(ncores):
        eidx = np.zeros((P, plan.sum_idx), dtype=np.int16)
        sdat = np.zeros((P, plan.sum_ch, P), dtype=ml_dtypes.bfloat16)
        iva = {}
        ivb = {}
        for t in range(nt):
            rows, ca, cb = plan.tiles[t]
            k = c * nt + t
            seg_src = g_src[bounds[k]:bounds[k + 1]]
            seg_rid = g_rid[bounds[k]:bounds[k + 1]]
            u = uniq[c, t]
            # assign flexible uniques to half A up to quota
            quotaA = min(ca * P, int(nAf[c, t] + nFl[c, t]))
            nA = max(int(nAf[c, t]), int(tot[c, t]) - cb * P)
            nA = min(quotaA, max(nA, int(nAf[c, t])))
            isA = np.zeros(len(u), dtype=bool)
            isA[u < B_OFF] = True
            flex_pos = np.nonzero((u >= B_OFF) & (u < A_END))[0]
            takeA = nA - int(nAf[c, t])
            if takeA > 0:
                isA[flex_pos[:takeA]] = True
            # slot for each unique source: A -> [0, nA), B -> [ca*P, ...)
            slot = np.zeros(len(u), dtype=np.int64)
            slot[isA] = np.arange(nA)
            slot[~isA] = ca * P + np.arange(len(u) - nA)
            # idx values (relative to half base), padded to chunks of 128
            iv = np.zeros(((ca + cb) * P,), dtype=np.int16)
            iv[slot[isA]] = u[isA].astype(np.int16)
            iv[slot[~isA]] = (u[~isA] - B_OFF).astype(np.int16)
            iva[t] = iv[:ca * P]
            ivb[t] = iv[ca * P:]
            # scatter-matrix: S[slot % 128, chunk, dst_row] += 1
            es = slot[np.searchsorted(u, seg_src)]
            mo = plan.meta_off[t]
            st32 = np.zeros((P, ca + cb, P), dtype=np.float32)
            np.add.at(st32, (es % P, es // P, seg_rid), 1.0)
            sdat[:, mo:mo + ca + cb, :] = st32.astype(ml_dtypes.bfloat16)
        # group idx layout: [A(t0) A(t1) .. B(t0) B(t1) ..], 16-wrapped
        for tlist, io, ca_tot, cb_tot in plan.groups:
            iv = np.concatenate([iva[t] for t in tlist]
                                + [ivb[t] for t in tlist])
            m = iv.reshape(-1, 16).T        # 16-wrap, replicate to 128
            eidx[:, io:io + (ca_tot + cb_tot) * 8] = np.tile(m, (8, 1))
        # per-node 1/deg for this core's dst rows: [P, nt]
        invc = np.zeros((nt * P,), dtype=np.float32)
        invc[:npc] = inv_node[c * npc:(c + 1) * npc]
        einv = np.ascontiguousarray(invc.reshape(nt, P).T)

        # self-chunk, transposed + tiled: xt[t, p, f, j] = Xc[t*128+j, f*128+p]
        xc = np.zeros((nt * P, D), dtype=ml_dtypes.bfloat16)
        xc[:npc] = feats_bf[c * npc:(c + 1) * npc]
        xt = np.ascontiguousarray(
            xc.reshape(nt, P, 4, P).transpose(0, 3, 2, 1))

        in_maps.append({
            "feats": feats_bf,
            "xt": xt,
            "wts": wts,
            "wtn": wtn,
            "bias": bias,
            "ones": ones,
            "ident": ident,
            "eidx": np.ascontiguousarray(eidx),
            "sdat": np.ascontiguousarray(sdat),
            "einv": einv,
        })
    return plan, in_maps


def build(plan):
    """Build + compile the SPMD Bass program for one core."""
    nc = bacc.Bacc("TRN2", target_bir_lowering=False, debug=False,
                   enable_asserts=False, num_devices=NCORES)
    n_nodes, npc = plan.n_nodes, plan.npc
    nt = len(plan.tiles)

    feats = nc.dram_tensor("feats", [n_nodes, D], BF16, kind="ExternalInput")
    xt = nc.dram_tensor("xt", [nt, P, 4, P], BF16, kind="ExternalInput")
    wts = nc.dram_tensor("wts", [P, 4, D], BF16, kind="ExternalInput")
    wtn = nc.dram_tensor("wtn", [P, 4, D], BF16, kind="ExternalInput")
    bias = nc.dram_tensor("bias", [1, D], BF16, kind="ExternalInput")
    ones = nc.dram_tensor("ones", [1, P], BF16, kind="ExternalInput")
    ident = nc.dram_tensor("ident", [P, P], BF16, kind="ExternalInput")
    eidx = nc.dram_tensor("eidx", [P, plan.sum_idx], I16, kind="ExternalInput")
    sdat = nc.dram_tensor("sdat", [P, plan.sum_ch, P], BF16,
                          kind="ExternalInput")
    einv = nc.dram_tensor("einv", [P, nt], F32, kind="ExternalInput")
    out = nc.dram_tensor("out", [npc, D], F32, kind="ExternalOutput")

    AF = mybir.ActivationFunctionType

    g0_idx = plan.groups[0][2] + plan.groups[0][3]   # group-0 idx columns

    with tile.TileContext(nc) as tc:
        with (
            tc.tile_pool(name="const", bufs=1) as cpool,
            tc.tile_pool(name="g", bufs=2) as gpool,
            tc.tile_pool(name="s", bufs=3) as spool,
            tc.tile_pool(name="x", bufs=2) as xpool,
            tc.tile_pool(name="h", bufs=2) as hpool,
            tc.tile_pool(name="ht", bufs=2) as htpool,
            tc.tile_pool(name="o", bufs=2) as opool,
            tc.tile_pool(name="ph", bufs=2, space="PSUM") as phpool,
            tc.tile_pool(name="ptr", bufs=2, space="PSUM") as ptrpool,
            tc.tile_pool(name="po", bufs=2, space="PSUM") as popool,
        ):
            # group-0 indices load first so the gather stream starts ASAP
            idx0_s = cpool.tile([P, g0_idx * 8], I16, tag="eidx0")
            nc.sync.dma_start(idx0_s[:], eidx[:, 0:g0_idx * 8])
            idx_s = cpool.tile([P, plan.sum_idx], I16, tag="eidx")
            nc.sync.dma_start(idx_s[:, g0_idx * 8:plan.sum_idx],
                              eidx[:, g0_idx * 8:plan.sum_idx])
            wts_s = cpool.tile([P, 4, D], BF16, tag="wts")
            nc.sync.dma_start(wts_s[:], wts[:])
            wtn_s = cpool.tile([P, 4, D], BF16, tag="wtn")
            nc.sync.dma_start(wtn_s[:], wtn[:])
            bias_s = cpool.tile([1, D], BF16, tag="bias")
            nc.sync.dma_start(bias_s[:], bias[:])
            ones_s = cpool.tile([1, P], BF16, tag="ones")
            nc.sync.dma_start(ones_s[:], ones[:])
            ident_s = cpool.tile([P, P], BF16, tag="ident")
            nc.sync.dma_start(ident_s[:], ident[:])
            inv_s = cpool.tile([P, nt], F32, tag="einv")
            nc.sync.dma_start(inv_s[:], einv[:])

            feats_a = feats[0:A_END, :]
            feats_b = feats[B_OFF:n_nodes, :]

            for gi, (tlist, io, ca_tot, cb_tot) in enumerate(plan.groups):
                gch = ca_tot + cb_tot
                g = gpool.tile([P, plan.gch_max, D], BF16, tag="g")
                isrc = idx0_s if gi == 0 else idx_s
                ib = 0 if gi == 0 else io
                for base, cn, src_ap in ((0, ca_tot, feats_a),
                                         (ca_tot, cb_tot, feats_b)):
                    for c0 in range(0, cn, GMAX):
                        cw = min(GMAX, cn - c0)
                        nc.gpsimd.dma_gather(
                            g[:, base + c0:base + c0 + cw, :], src_ap,
                            isrc[:, ib + (base + c0) * 8:
                                 ib + (base + c0 + cw) * 8],
                            cw * P, cw * P, D, single_packet=False)

                for t in tlist:
                    rows, ca, cb = plan.tiles[t]
                    ch = ca + cb
                    mo = plan.meta_off[t]
                    cols = plan.gcols[t]

                    s = spool.tile([P, plan.ch_max, P], BF16, tag="s")
                    nc.sync.dma_start(s[:, 0:ch, :], sdat[:, mo:mo + ch, :])

                    xt_t = xpool.tile([P, 4, P], BF16, tag="x")
                    nc.sync.dma_start(xt_t[:], xt[t])

                    # aggregation: psum_h[dst, feat] += S_c.T @ G_c
                    ph = phpool.tile([P, D], F32, tag="ph")
                    for c in range(ch):
                        nc.tensor.matmul(
                            ph[:], s[:, c, :], g[:, cols[c], :],
                            start=(c == 0), stop=(c == ch - 1))

                    # h = ph * (1/deg): per-partition scale, PSUM->SBUF on
                    # the scalar engine (DVE would stall behind GpSimd).
                    h = hpool.tile([P, D], BF16, tag="h")
                    nc.scalar.activation(h[:], ph[:], AF.Copy,
                                         scale=inv_s[:, t:t + 1])

                    # transpose h -> hT as plain matmuls against identity
                    ht = htpool.tile([P, 4, P], BF16, tag="ht")
                    ptr = ptrpool.tile([P, 4, P], F32, tag="ptr")
                    for f in range(4):
                        nc.tensor.matmul(
                            ptr[:, f, :], h[:, f * P:(f + 1) * P],
                            ident_s[:], start=True, stop=True)
                    nc.scalar.activation(ht[:], ptr[:], AF.Copy)

                    # out = relu(bias + X @ Wself.T + h @ Wneigh.T)
                    po = popool.tile([P, D], F32, tag="po")
                    nc.tensor.matmul(po[:], ones_s[:], bias_s[:],
                                     start=True, stop=False)
                    for f in range(4):
                        nc.tensor.matmul(po[:], xt_t[:, f, :],
                                         wts_s[:, f, :],
                                         start=False, stop=False)
                        nc.tensor.matmul(po[:], ht[:, f, :],
                                         wtn_s[:, f, :],
                                         start=False, stop=(f == 3))

                    o = opool.tile([P, D], F32, tag="o")
                    nc.scalar.activation(o[:], po[:], AF.Relu)
                    nc.sync.dma_start(out[t * P:t * P + rows, :],
                                      o[:rows, :])

    nc.compile()
    return nc


_cache = {}


def _get_nc(plan):
    k = plan.key()
    if k not in _cache:
        _cache[k] = build(plan)
    return _cache[k]


def kernel(local_feats, src, dst, layer=None, W_self=None, W_neigh=None,
           b=None, **_unused):
    plan, in_maps = _prepare(local_feats, src, dst, W_self, W_neigh, b)
    nc = _get_nc(plan)
    res = run_bass_kernel_spmd(nc, in_maps, core_ids=list(range(NCORES)))
    return np.concatenate([res.results[c]["out"] for c in range(NCORES)],
                          axis=0)


# revision 11
# speedup vs baseline: 1.2945x; 1.0024x over previous
"""DistSAGEConv forward on 8 Trainium2 NeuronCores (Bass/Tile), bf16 compute.

Math (matches the reference):
    h_neigh = segment_mean(local_feats[src], dst)            # [N, D]
    out     = relu(local_feats @ W_self.T + h_neigh @ W_neigh.T + b)

Distribution: nodes (and their incident dst edges) are sharded across the 8
cores, 6250 nodes each; the weights/bias are replicated; the full feature
table is replicated into every core's HBM so "remote neighbor features" are
indirect-DMA gathers from the local copy (the halo exchange of the Dist
semantics collapses to a local gather because we receive full inputs).

The kernel is bound by SWDGE descriptor generation on the GpSimd engine
(~9 ns per gathered row), so the design minimizes gathered rows and keeps
every other engine strictly below that wall:

  1. Sources are deduplicated per dst-tile and the int16-indexed table is
     addressed as two OVERLAPPING halves A=[0,32768) / B=[17232,50000);
     edges in the overlap are assigned host-side to equalize chunk counts
     across cores and minimize 128-padding.
  2. The scatter matrices S[slot, dst] (multiplicity counts, bf16-exact)
     are precomputed on host and DMAed in via HWDGE -- the vector engine
     does no work at all (DVE contends with GpSimd for an SBUF port, so
     any DVE op stalls behind the descriptor stream).
  3. psum_h[128 dst, 512] += S_c.T @ G_c per 128-slot chunk on the tensor
     engine; 1/deg is applied as a per-partition scale during the
     PSUM->SBUF copy on the *scalar* engine (ACT), as are the hT copies.
  4. hT via PE transposes; out = relu(bias + X @ Wself.T + h @ Wneigh.T).

Edge bookkeeping (tile/half assignment, dedup, S construction, degrees,
padding) is integer preprocessing done on host with numpy; all
floating-point math happens on device (inputs cast to bf16, output fp32).
"""

import ml_dtypes
import numpy as np

from concourse import bass, bacc, mybir, tile
from concourse.bass_utils import run_bass_kernel_spmd

F32 = mybir.dt.float32
BF16 = mybir.dt.bfloat16
I16 = mybir.dt.int16

N_NODES = 50000
N_EDGES = 800000
D = 512
NCORES = 8
NPC = N_NODES // NCORES          # 6250 nodes per core
P = 128                          # partitions / tile rows
NT = (NPC + P - 1) // P          # 49 dst tiles per core (last has 106 rows)
A_END = 32768                    # half A covers [0, A_END)
B_OFF = N_NODES - 32768          # half B covers [B_OFF, N_NODES)
GMAX = 20                        # chunks per dma_gather call (2560 idx ok
                                 # with single_packet=False; single-packet
                                 # calls wedge at >=1024 idx = 64 desc/eng)
NTG = 4                          # dst tiles gathered per call group


class Plan:
    """Compile-time structure shared by all 8 cores (program is SPMD).

    Tiles are gathered in groups of NTG: one gather call per table half
    per group (descriptor generation on the Q7s is the kernel's wall, so
    fewer/bigger calls win).  Within a group's G buffer the chunk columns
    are laid out [A(t0) A(t1) .. B(t0) B(t1) ..]; gcols maps each tile's
    local chunks to its G columns.  sdat stays per-tile contiguous.
    """

    def __init__(self, n_nodes, npc, tiles):
        self.n_nodes = n_nodes
        self.npc = npc
        self.tiles = tiles           # per tile: (rows, cA, cB)
        nt = len(tiles)
        self.meta_off = []
        mo = 0
        for _, ca, cb in tiles:
            self.meta_off.append(mo)
            mo += ca + cb
        self.sum_ch = mo
        self.ch_max = max(ca + cb for _, ca, cb in tiles)

        self.groups = []             # (tlist, idx_off, ca_tot, cb_tot)
        self.gcols = {}              # tile -> list of G columns (len ca+cb)
        self.gch_max = 0
        io = 0
        for g0 in range(0, nt, NTG):
            tlist = list(range(g0, min(g0 + NTG, nt)))
            ca_tot = sum(tiles[t][1] for t in tlist)
            cb_tot = sum(tiles[t][2] for t in tlist)
            aoff = 0
            boff = ca_tot
            for t in tlist:
                _, ca, cb = tiles[t]
                self.gcols[t] = (list(range(aoff, aoff + ca))
                                 + list(range(boff, boff + cb)))
                aoff += ca
                boff += cb
            self.groups.append((tlist, io, ca_tot, cb_tot))
            io += (ca_tot + cb_tot) * 8      # int16 idx columns (16-wrap)
            self.gch_max = max(self.gch_max, ca_tot + cb_tot)
        self.sum_idx = io

    def key(self):
        return (self.n_nodes, self.npc, tuple(self.tiles))


def _prepare(local_feats, src, dst, W_self, W_neigh, b,
             n_nodes=N_NODES, ncores=NCORES):
    """Host-side integer preprocessing -> (plan, in_maps)."""
    npc = n_nodes // ncores
    nt = (npc + P - 1) // P
    feats_bf = np.ascontiguousarray(
        np.asarray(local_feats, dtype=np.float32).astype(ml_dtypes.bfloat16))
    src = np.asarray(src).astype(np.int64)
    dst = np.asarray(dst).astype(np.int64)

    deg = np.bincount(dst, minlength=n_nodes).astype(np.float32)
    inv_node = (1.0 / np.maximum(deg, 1.0)).astype(np.float32)

    # group edges by (core, tile); within a tile dedup sources and count
    # multiplicity per (source, dst-row)
    core_of = dst // npc
    local = dst - core_of * npc
    t_of = local // P
    r_of = local % P
    gkey = core_of * nt + t_of
    order = np.argsort(gkey, kind="stable")
    g_src = src[order]
    g_rid = r_of[order]
    bounds = np.searchsorted(gkey[order], np.arange(ncores * nt + 1))

    # pass 1: per (core, tile) unique sources split into forced-A / forced-B
    # / flexible; derive shared chunk counts (ca, cb) per tile.
    uniq = {}
    nAf = np.zeros((ncores, nt), dtype=np.int64)
    nBf = np.zeros((ncores, nt), dtype=np.int64)
    nFl = np.zeros((ncores, nt), dtype=np.int64)
    for c in range(ncores):
        for t in range(nt):
            k = c * nt + t
            u = np.unique(g_src[bounds[k]:bounds[k + 1]])
            uniq[c, t] = u
            nAf[c, t] = np.count_nonzero(u < B_OFF)
            nBf[c, t] = np.count_nonzero(u >= A_END)
            nFl[c, t] = len(u) - nAf[c, t] - nBf[c, t]
    tot = nAf + nBf + nFl

    tiles = []
    for t in range(nt):
        rows = min(P, npc - t * P)
        need = int(np.max(np.ceil(tot[:, t] / P)))
        ca_lo = int(np.max(np.ceil(nAf[:, t] / P)))
        cb_lo = int(np.max(np.ceil(nBf[:, t] / P)))
        while True:
            ok = None
            for ca in range(ca_lo, need - cb_lo + 1):
                cb = need - ca
                if np.all(nAf[:, t] + nFl[:, t] >= tot[:, t] - cb * P):
                    ok = (ca, cb)
                    break
            if ok is not None:
                break
            need += 1
        tiles.append((rows, ok[0], ok[1]))
    plan = Plan(n_nodes, npc, tiles)

    # replicated constants (bf16)
    wts = np.ascontiguousarray(
        W_self.T.astype(ml_dtypes.bfloat16).reshape(4, P, D).transpose(1, 0, 2))
    wtn = np.ascontiguousarray(
        W_neigh.T.astype(ml_dtypes.bfloat16).reshape(4, P, D).transpose(1, 0, 2))
    bias = np.ascontiguousarray(b.astype(ml_dtypes.bfloat16).reshape(1, D))
    ones = np.ones((1, P), dtype=ml_dtypes.bfloat16)
    ident = np.eye(P, dtype=ml_dtypes.bfloat16)

    in_maps = []
    for c in range(ncores):
        eidx = np.zeros((P, plan.sum_idx), dtype=np.int16)
        sdat = np.zeros((P, plan.sum_ch, P), dtype=ml_dtypes.bfloat16)
        iva = {}
        ivb = {}
        for t in range(nt):
            rows, ca, cb = plan.tiles[t]
            k = c * nt + t
            seg_src = g_src[bounds[k]:bounds[k + 1]]
            seg_rid = g_rid[bounds[k]:bounds[k + 1]]
            u = uniq[c, t]
            # assign flexible uniques to half A up to quota
            quotaA = min(ca * P, int(nAf[c, t] + nFl[c, t]))
            nA = max(int(nAf[c, t]), int(tot[c, t]) - cb * P)
            nA = min(quotaA, max(nA, int(nAf[c, t])))
            isA = np.zeros(len(u), dtype=bool)
            isA[u < B_OFF] = True
            flex_pos = np.nonzero((u >= B_OFF) & (u < A_END))[0]
            takeA = nA - int(nAf[c, t])
            if takeA > 0:
                isA[flex_pos[:takeA]] = True
            # slot for each unique source: A -> [0, nA), B -> [ca*P, ...)
            slot = np.zeros(len(u), dtype=np.int64)
            slot[isA] = np.arange(nA)
            slot[~isA] = ca * P + np.arange(len(u) - nA)
            # idx values (relative to half base), padded to chunks of 128
            iv = np.zeros(((ca + cb) * P,), dtype=np.int16)
            iv[slot[isA]] = u[isA].astype(np.int16)
            iv[slot[~isA]] = (u[~isA] - B_OFF).astype(np.int16)
            iva[t] = iv[:ca * P]
            ivb[t] = iv[ca * P:]
            # scatter-matrix: S[slot % 128, chunk, dst_row] += 1
            es = slot[np.searchsorted(u, seg_src)]
            mo = plan.meta_off[t]
            st32 = np.zeros((P, ca + cb, P), dtype=np.float32)
            np.add.at(st32, (es % P, es // P, seg_rid), 1.0)
            sdat[:, mo:mo + ca + cb, :] = st32.astype(ml_dtypes.bfloat16)
        # group idx layout: [A(t0) A(t1) .. B(t0) B(t1) ..], 16-wrapped
        for tlist, io, ca_tot, cb_tot in plan.groups:
            iv = np.concatenate([iva[t] for t in tlist]
                                + [ivb[t] for t in tlist])
            m = iv.reshape(-1, 16).T        # 16-wrap, replicate to 128
            eidx[:, io:io + (ca_tot + cb_tot) * 8] = np.tile(m, (8, 1))
        # per-node 1/deg for this core's dst rows: [P, nt]
        invc = np.zeros((nt * P,), dtype=np.float32)
        invc[:npc] = inv_node[c * npc:(c + 1) * npc]
        einv = np.ascontiguousarray(invc.reshape(nt, P).T)

        # self-chunk, transposed + tiled: xt[t, p, f, j] = Xc[t*128+j, f*128+p]
        xc = np.zeros((nt * P, D), dtype=ml_dtypes.bfloat16)
        xc[:npc] = feats_bf[c * npc:(c + 1) * npc]
        xt = np.ascontiguousarray(
            xc.reshape(nt, P, 4, P).transpose(0, 3, 2, 1))

        in_maps.append({
            "feats": feats_bf,
            "xt": xt,
            "wts": wts,
            "wtn": wtn,
            "bias": bias,
            "ones": ones,
            "ident": ident,
            "eidx": np.ascontiguousarray(eidx),
            "sdat": np.ascontiguousarray(sdat),
            "einv": einv,
        })
    return plan, in_maps


def build(plan):
    """Build + compile the SPMD Bass program for one core."""
    nc = bacc.Bacc("TRN2", target_bir_lowering=False, debug=False,
                   enable_asserts=False, num_devices=NCORES)
    n_nodes, npc = plan.n_nodes, plan.npc
    nt = len(plan.tiles)

    feats = nc.dram_tensor("feats", [n_nodes, D], BF16, kind="ExternalInput")
    xt = nc.dram_tensor("xt", [nt, P, 4, P], BF16, kind="ExternalInput")
    wts = nc.dram_tensor("wts", [P, 4, D], BF16, kind="ExternalInput")
    wtn = nc.dram_tensor("wtn", [P, 4, D], BF16, kind="ExternalInput")
    bias = nc.dram_tensor("bias", [1, D], BF16, kind="ExternalInput")
    ones = nc.dram_tensor("ones", [1, P], BF16, kind="ExternalInput")
    ident = nc.dram_tensor("ident", [P, P], BF16, kind="ExternalInput")
    eidx = nc.dram_tensor("eidx", [P, plan.sum_idx], I16, kind="ExternalInput")
    sdat = nc.dram_tensor("sdat", [P, plan.sum_ch, P], BF16,
                          kind="ExternalInput")
    einv = nc.dram_tensor("einv", [P, nt], F32, kind="ExternalInput")
    out = nc.dram_tensor("out", [npc, D], F32, kind="ExternalOutput")

    AF = mybir.ActivationFunctionType

    g0_idx = plan.groups[0][2] + plan.groups[0][3]   # group-0 idx columns

    with tile.TileContext(nc) as tc:
        with (
            tc.tile_pool(name="const", bufs=1) as cpool,
            tc.tile_pool(name="g", bufs=2) as gpool,
            tc.tile_pool(name="s", bufs=3) as spool,
            tc.tile_pool(name="x", bufs=2) as xpool,
            tc.tile_pool(name="h", bufs=2) as hpool,
            tc.tile_pool(name="ht", bufs=2) as htpool,
            tc.tile_pool(name="o", bufs=2) as opool,
            tc.tile_pool(name="ph", bufs=2, space="PSUM") as phpool,
            tc.tile_pool(name="ptr", bufs=2, space="PSUM") as ptrpool,
            tc.tile_pool(name="po", bufs=2, space="PSUM") as popool,
        ):
            # group-0 indices load first so the gather stream starts ASAP
            idx0_s = cpool.tile([P, g0_idx * 8], I16, tag="eidx0")
            nc.sync.dma_start(idx0_s[:], eidx[:, 0:g0_idx * 8])
            idx_s = cpool.tile([P, plan.sum_idx], I16, tag="eidx")
            nc.sync.dma_start(idx_s[:, g0_idx * 8:plan.sum_idx],
                              eidx[:, g0_idx * 8:plan.sum_idx])
            wts_s = cpool.tile([P, 4, D], BF16, tag="wts")
            nc.sync.dma_start(wts_s[:], wts[:])
            wtn_s = cpool.tile([P, 4, D], BF16, tag="wtn")
            nc.sync.dma_start(wtn_s[:], wtn[:])
            bias_s = cpool.tile([1, D], BF16, tag="bias")
            nc.sync.dma_start(bias_s[:], bias[:])
            ones_s = cpool.tile([1, P], BF16, tag="ones")
            nc.sync.dma_start(ones_s[:], ones[:])
            ident_s = cpool.tile([P, P], BF16, tag="ident")
            nc.sync.dma_start(ident_s[:], ident[:])
            inv_s = cpool.tile([P, nt], F32, tag="einv")
            nc.sync.dma_start(inv_s[:], einv[:])

            feats_a = feats[0:A_END, :]
            feats_b = feats[B_OFF:n_nodes, :]

            for gi, (tlist, io, ca_tot, cb_tot) in enumerate(plan.groups):
                gch = ca_tot + cb_tot
                g = gpool.tile([P, plan.gch_max, D], BF16, tag="g")
                isrc = idx0_s if gi == 0 else idx_s
                ib = 0 if gi == 0 else io
                for base, cn, src_ap in ((0, ca_tot, feats_a),
                                         (ca_tot, cb_tot, feats_b)):
                    for c0 in range(0, cn, GMAX):
                        cw = min(GMAX, cn - c0)
                        nc.gpsimd.dma_gather(
                            g[:, base + c0:base + c0 + cw, :], src_ap,
                            isrc[:, ib + (base + c0) * 8:
                                 ib + (base + c0 + cw) * 8],
                            cw * P, cw * P, D, single_packet=False)

                for t in tlist:
                    rows, ca, cb = plan.tiles[t]
                    ch = ca + cb
                    mo = plan.meta_off[t]
                    cols = plan.gcols[t]

                    s = spool.tile([P, plan.ch_max, P], BF16, tag="s")
                    nc.sync.dma_start(s[:, 0:ch, :], sdat[:, mo:mo + ch, :])

                    xt_t = xpool.tile([P, 4, P], BF16, tag="x")
                    nc.sync.dma_start(xt_t[:], xt[t])

                    # aggregation: psum_h[dst, feat] += S_c.T @ G_c
                    ph = phpool.tile([P, D], F32, tag="ph")
                    for c in range(ch):
                        nc.tensor.matmul(
                            ph[:], s[:, c, :], g[:, cols[c], :],
                            start=(c == 0), stop=(c == ch - 1))

                    # h = ph * (1/deg): per-partition scale, PSUM->SBUF on
                    # the scalar engine (DVE would stall behind GpSimd).
                    h = hpool.tile([P, D], BF16, tag="h")
                    nc.scalar.activation(h[:], ph[:], AF.Copy,
                                         scale=inv_s[:, t:t + 1])

                    # transpose h -> hT as plain matmuls against identity
                    ht = htpool.tile([P, 4, P], BF16, tag="ht")
                    ptr = ptrpool.tile([P, 4, P], F32, tag="ptr")
                    for f in range(4):
                        nc.tensor.matmul(
                            ptr[:, f, :], h[:, f * P:(f + 1) * P],
                            ident_s[:], start=True, stop=True)
                    nc.scalar.activation(ht[:], ptr[:], AF.Copy)

                    # out = relu(bias + X @ Wself.T + h @ Wneigh.T)
                    po = popool.tile([P, D], F32, tag="po")
                    nc.tensor.matmul(po[:], ones_s[:], bias_s[:],
                                     start=True, stop=False)
                    for f in range(4):
                        nc.tensor.matmul(po[:], xt_t[:, f, :],
                                         wts_s[:, f, :],
                                         start=False, stop=False)
                        nc.tensor.matmul(po[:], ht[:, f, :],
                                         wtn_s[:, f, :],
                                         start=False, stop=(f == 3))

                    o = opool.tile([P, D], F32, tag="o")
                    nc.scalar.activation(o[:], po[:], AF.Relu)
                    nc.sync.dma_start(out[t * P:t * P + rows, :],
                                      o[:rows, :])

    nc.compile()
    return nc


_cache = {}


def _get_nc(plan):
    k = plan.key()
    if k not in _cache:
        _cache[k] = build(plan)
    return _cache[k]


def kernel(local_feats, src, dst, layer=None, W_self=None, W_neigh=None,
           b=None, **_unused):
    plan, in_maps = _prepare(local_feats, src, dst, W_self, W_neigh, b)
    nc = _get_nc(plan)
    res = run_bass_kernel_spmd(nc, in_maps, core_ids=list(range(NCORES)))
    return np.concatenate([res.results[c]["out"] for c in range(NCORES)],
                          axis=0)


# revision 14
# speedup vs baseline: 1.2994x; 1.0038x over previous
"""DistSAGEConv forward on 8 Trainium2 NeuronCores (Bass/Tile), bf16 compute.

Math (matches the reference):
    h_neigh = segment_mean(local_feats[src], dst)            # [N, D]
    out     = relu(local_feats @ W_self.T + h_neigh @ W_neigh.T + b)

Distribution: nodes (and their incident dst edges) are sharded across the 8
cores, 6250 nodes each; the weights/bias are replicated; the full feature
table is replicated into every core's HBM so "remote neighbor features" are
indirect-DMA gathers from the local copy (the halo exchange of the Dist
semantics collapses to a local gather because we receive full inputs).

The kernel is bound by SWDGE descriptor generation on the GpSimd engine
(~9 ns per gathered row), so the design minimizes gathered rows and keeps
every other engine strictly below that wall:

  1. Sources are deduplicated per dst-tile and the int16-indexed table is
     addressed as two OVERLAPPING halves A=[0,32768) / B=[17232,50000);
     edges in the overlap are assigned host-side to equalize chunk counts
     across cores and minimize 128-padding.
  2. The scatter matrices S[slot, dst] (multiplicity counts, bf16-exact)
     are precomputed on host and DMAed in via HWDGE -- the vector engine
     does no work at all (DVE contends with GpSimd for an SBUF port, so
     any DVE op stalls behind the descriptor stream).
  3. psum_h[128 dst, 512] += S_c.T @ G_c per 128-slot chunk on the tensor
     engine; 1/deg is applied as a per-partition scale during the
     PSUM->SBUF copy on the *scalar* engine (ACT), as are the hT copies.
  4. hT via PE transposes; out = relu(bias + X @ Wself.T + h @ Wneigh.T).

Edge bookkeeping (tile/half assignment, dedup, S construction, degrees,
padding) is integer preprocessing done on host with numpy; all
floating-point math happens on device (inputs cast to bf16, output fp32).
"""

import ml_dtypes
import numpy as np

from concourse import bass, bacc, library_config, mybir, tile
from concourse.bass_utils import run_bass_kernel_spmd

F32 = mybir.dt.float32
BF16 = mybir.dt.bfloat16
I16 = mybir.dt.int16

N_NODES = 50000
N_EDGES = 800000
D = 512
NCORES = 8
NPC = N_NODES // NCORES          # 6250 nodes per core
P = 128                          # partitions / tile rows
NT = (NPC + P - 1) // P          # 49 dst tiles per core (last has 106 rows)
A_END = 32768                    # half A covers [0, A_END)
B_OFF = N_NODES - 32768          # half B covers [B_OFF, N_NODES)
GMAX = 36                        # chunks per dma_gather call (4608 idx ok
                                 # with single_packet=False; single-packet
                                 # calls wedge at >=1024 idx = 64 desc/eng)
NTG = 4                          # dst tiles gathered per call group


class Plan:
    """Compile-time structure shared by all 8 cores (program is SPMD).

    Tiles are gathered in groups of NTG: one gather call per table half
    per group (descriptor generation on the Q7s is the kernel's wall, so
    fewer/bigger calls win).  Within a group's G buffer the chunk columns
    are laid out [A(t0) A(t1) .. B(t0) B(t1) ..]; gcols maps each tile's
    local chunks to its G columns.  sdat stays per-tile contiguous.
    """

    def __init__(self, n_nodes, npc, tiles):
        self.n_nodes = n_nodes
        self.npc = npc
        self.tiles = tiles           # per tile: (rows, cA, cB)
        nt = len(tiles)
        self.meta_off = []
        mo = 0
        for _, ca, cb in tiles:
            self.meta_off.append(mo)
            mo += ca + cb
        self.sum_ch = mo
        self.ch_max = max(ca + cb for _, ca, cb in tiles)

        self.groups = []             # (tlist, idx_off, ca_tot, cb_tot)
        self.gcols = {}              # tile -> list of G columns (len ca+cb)
        self.gch_max = 0
        io = 0
        for g0 in range(0, nt, NTG):
            tlist = list(range(g0, min(g0 + NTG, nt)))
            ca_tot = sum(tiles[t][1] for t in tlist)
            cb_tot = sum(tiles[t][2] for t in tlist)
            aoff = 0
            boff = ca_tot
            for t in tlist:
                _, ca, cb = tiles[t]
                self.gcols[t] = (list(range(aoff, aoff + ca))
                                 + list(range(boff, boff + cb)))
                aoff += ca
                boff += cb
            self.groups.append((tlist, io, ca_tot, cb_tot))
            io += (ca_tot + cb_tot) * 8      # int16 idx columns (16-wrap)
            self.gch_max = max(self.gch_max, ca_tot + cb_tot)
        self.sum_idx = io

    def key(self):
        return (self.n_nodes, self.npc, tuple(self.tiles))


def _prepare(local_feats, src, dst, W_self, W_neigh, b,
             n_nodes=N_NODES, ncores=NCORES):
    """Host-side integer preprocessing -> (plan, in_maps)."""
    npc = n_nodes // ncores
    nt = (npc + P - 1) // P
    feats_bf = np.ascontiguousarray(
        np.asarray(local_feats, dtype=np.float32).astype(ml_dtypes.bfloat16))
    src = np.asarray(src).astype(np.int64)
    dst = np.asarray(dst).astype(np.int64)

    deg = np.bincount(dst, minlength=n_nodes).astype(np.float32)
    inv_node = (1.0 / np.maximum(deg, 1.0)).astype(np.float32)

    # group edges by (core, tile); within a tile dedup sources and count
    # multiplicity per (source, dst-row)
    core_of = dst // npc
    local = dst - core_of * npc
    t_of = local // P
    r_of = local % P
    gkey = core_of * nt + t_of
    order = np.argsort(gkey, kind="stable")
    g_src = src[order]
    g_rid = r_of[order]
    bounds = np.searchsorted(gkey[order], np.arange(ncores * nt + 1))

    # pass 1: per (core, tile) unique sources split into forced-A / forced-B
    # / flexible; derive shared chunk counts (ca, cb) per tile.
    uniq = {}
    nAf = np.zeros((ncores, nt), dtype=np.int64)
    nBf = np.zeros((ncores, nt), dtype=np.int64)
    nFl = np.zeros((ncores, nt), dtype=np.int64)
    for c in range(ncores):
        for t in range(nt):
            k = c * nt + t
            u = np.unique(g_src[bounds[k]:bounds[k + 1]])
            uniq[c, t] = u
            nAf[c, t] = np.count_nonzero(u < B_OFF)
            nBf[c, t] = np.count_nonzero(u >= A_END)
            nFl[c, t] = len(u) - nAf[c, t] - nBf[c, t]
    tot = nAf + nBf + nFl

    tiles = []
    for t in range(nt):
        rows = min(P, npc - t * P)
        need = int(np.max(np.ceil(tot[:, t] / P)))
        ca_lo = int(np.max(np.ceil(nAf[:, t] / P)))
        cb_lo = int(np.max(np.ceil(nBf[:, t] / P)))
        while True:
            ok = None
            for ca in range(ca_lo, need - cb_lo + 1):
                cb = need - ca
                if np.all(nAf[:, t] + nFl[:, t] >= tot[:, t] - cb * P):
                    ok = (ca, cb)
                    break
            if ok is not None:
                break
            need += 1
        tiles.append((rows, ok[0], ok[1]))
    plan = Plan(n_nodes, npc, tiles)

    # replicated constants (bf16)
    wts = np.ascontiguousarray(
        W_self.T.astype(ml_dtypes.bfloat16).reshape(4, P, D).transpose(1, 0, 2))
    wtn = np.ascontiguousarray(
        W_neigh.T.astype(ml_dtypes.bfloat16).reshape(4, P, D).transpose(1, 0, 2))
    bias = np.ascontiguousarray(b.astype(ml_dtypes.bfloat16).reshape(1, D))
    ones = np.ones((1, P), dtype=ml_dtypes.bfloat16)
    ident = np.eye(P, dtype=ml_dtypes.bfloat16)

    in_maps = []
    for c in range(ncores):
        eidx = np.zeros((P, plan.sum_idx), dtype=np.int16)
        sdat = np.zeros((P, plan.sum_ch, P), dtype=ml_dtypes.bfloat16)
        iva = {}
        ivb = {}
        for t in range(nt):
            rows, ca, cb = plan.tiles[t]
            k = c * nt + t
            seg_src = g_src[bounds[k]:bounds[k + 1]]
            seg_rid = g_rid[bounds[k]:bounds[k + 1]]
            u = uniq[c, t]
            # assign flexible uniques to half A up to quota
            quotaA = min(ca * P, int(nAf[c, t] + nFl[c, t]))
            nA = max(int(nAf[c, t]), int(tot[c, t]) - cb * P)
            nA = min(quotaA, max(nA, int(nAf[c, t])))
            isA = np.zeros(len(u), dtype=bool)
            isA[u < B_OFF] = True
            flex_pos = np.nonzero((u >= B_OFF) & (u < A_END))[0]
            takeA = nA - int(nAf[c, t])
            if takeA > 0:
                isA[flex_pos[:takeA]] = True
            # slot for each unique source: A -> [0, nA), B -> [ca*P, ...)
            slot = np.zeros(len(u), dtype=np.int64)
            slot[isA] = np.arange(nA)
            slot[~isA] = ca * P + np.arange(len(u) - nA)
            # idx values (relative to half base), padded to chunks of 128
            iv = np.zeros(((ca + cb) * P,), dtype=np.int16)
            iv[slot[isA]] = u[isA].astype(np.int16)
            iv[slot[~isA]] = (u[~isA] - B_OFF).astype(np.int16)
            iva[t] = iv[:ca * P]
            ivb[t] = iv[ca * P:]
            # scatter-matrix: S[slot % 128, chunk, dst_row] += 1
            es = slot[np.searchsorted(u, seg_src)]
            mo = plan.meta_off[t]
            st32 = np.zeros((P, ca + cb, P), dtype=np.float32)
            np.add.at(st32, (es % P, es // P, seg_rid), 1.0)
            sdat[:, mo:mo + ca + cb, :] = st32.astype(ml_dtypes.bfloat16)
        # group idx layout: [A(t0) A(t1) .. B(t0) B(t1) ..], 16-wrapped
        for tlist, io, ca_tot, cb_tot in plan.groups:
            iv = np.concatenate([iva[t] for t in tlist]
                                + [ivb[t] for t in tlist])
            m = iv.reshape(-1, 16).T        # 16-wrap, replicate to 128
            eidx[:, io:io + (ca_tot + cb_tot) * 8] = np.tile(m, (8, 1))
        # per-node 1/deg for this core's dst rows: [P, nt]
        invc = np.zeros((nt * P,), dtype=np.float32)
        invc[:npc] = inv_node[c * npc:(c + 1) * npc]
        einv = np.ascontiguousarray(invc.reshape(nt, P).T)

        # self-chunk, transposed + tiled: xt[t, p, f, j] = Xc[t*128+j, f*128+p]
        xc = np.zeros((nt * P, D), dtype=ml_dtypes.bfloat16)
        xc[:npc] = feats_bf[c * npc:(c + 1) * npc]
        xt = np.ascontiguousarray(
            xc.reshape(nt, P, 4, P).transpose(0, 3, 2, 1))

        in_maps.append({
            "feats": feats_bf,
            "xt": xt,
            "wts": wts,
            "wtn": wtn,
            "bias": bias,
            "ones": ones,
            "ident": ident,
            "eidx": np.ascontiguousarray(eidx),
            "sdat": np.ascontiguousarray(sdat),
            "einv": einv,
        })
    return plan, in_maps


def build(plan):
    """Build + compile the SPMD Bass program for one core."""
    nc = bacc.Bacc("TRN2", target_bir_lowering=False, debug=False,
                   enable_asserts=False, num_devices=NCORES)
    n_nodes, npc = plan.n_nodes, plan.npc
    nt = len(plan.tiles)

    feats = nc.dram_tensor("feats", [n_nodes, D], BF16, kind="ExternalInput")
    xt = nc.dram_tensor("xt", [nt, P, 4, P], BF16, kind="ExternalInput")
    wts = nc.dram_tensor("wts", [P, 4, D], BF16, kind="ExternalInput")
    wtn = nc.dram_tensor("wtn", [P, 4, D], BF16, kind="ExternalInput")
    bias = nc.dram_tensor("bias", [1, D], BF16, kind="ExternalInput")
    ones = nc.dram_tensor("ones", [1, P], BF16, kind="ExternalInput")
    ident = nc.dram_tensor("ident", [P, P], BF16, kind="ExternalInput")
    eidx = nc.dram_tensor("eidx", [P, plan.sum_idx], I16, kind="ExternalInput")
    sdat = nc.dram_tensor("sdat", [P, plan.sum_ch, P], BF16,
                          kind="ExternalInput")
    einv = nc.dram_tensor("einv", [P, nt], F32, kind="ExternalInput")
    out = nc.dram_tensor("out", [npc, D], F32, kind="ExternalOutput")

    AF = mybir.ActivationFunctionType

    g0_idx = plan.groups[0][2] + plan.groups[0][3]   # group-0 idx columns

    with tile.TileContext(nc) as tc:
        with (
            tc.tile_pool(name="const", bufs=1) as cpool,
            tc.tile_pool(name="g", bufs=2) as gpool,
            tc.tile_pool(name="s", bufs=3) as spool,
            tc.tile_pool(name="x", bufs=2) as xpool,
            tc.tile_pool(name="h", bufs=2) as hpool,
            tc.tile_pool(name="ht", bufs=2) as htpool,
            tc.tile_pool(name="o", bufs=2) as opool,
            tc.tile_pool(name="ph", bufs=2, space="PSUM") as phpool,
            tc.tile_pool(name="ptr", bufs=2, space="PSUM") as ptrpool,
            tc.tile_pool(name="po", bufs=2, space="PSUM") as popool,
        ):
            # preload the gather ucode library so its IRAM DMA overlaps the
            # preamble instead of stalling the first dma_gather
            nc.gpsimd.load_library(library_config.mlp)
            # group-0 indices load first so the gather stream starts ASAP
            idx0_s = cpool.tile([P, g0_idx * 8], I16, tag="eidx0")
            nc.sync.dma_start(idx0_s[:], eidx[:, 0:g0_idx * 8])
            idx_s = cpool.tile([P, plan.sum_idx], I16, tag="eidx")
            nc.sync.dma_start(idx_s[:, g0_idx * 8:plan.sum_idx],
                              eidx[:, g0_idx * 8:plan.sum_idx])
            wts_s = cpool.tile([P, 4, D], BF16, tag="wts")
            nc.sync.dma_start(wts_s[:], wts[:])
            wtn_s = cpool.tile([P, 4, D], BF16, tag="wtn")
            nc.sync.dma_start(wtn_s[:], wtn[:])
            bias_s = cpool.tile([1, D], BF16, tag="bias")
            nc.sync.dma_start(bias_s[:], bias[:])
            ones_s = cpool.tile([1, P], BF16, tag="ones")
            nc.sync.dma_start(ones_s[:], ones[:])
            ident_s = cpool.tile([P, P], BF16, tag="ident")
            nc.sync.dma_start(ident_s[:], ident[:])
            inv_s = cpool.tile([P, nt], F32, tag="einv")
            nc.sync.dma_start(inv_s[:], einv[:])

            feats_a = feats[0:A_END, :]
            feats_b = feats[B_OFF:n_nodes, :]

            for gi, (tlist, io, ca_tot, cb_tot) in enumerate(plan.groups):
                gch = ca_tot + cb_tot
                g = gpool.tile([P, plan.gch_max, D], BF16, tag="g")
                isrc = idx0_s if gi == 0 else idx_s
                ib = 0 if gi == 0 else io
                for base, cn, src_ap in ((0, ca_tot, feats_a),
                                         (ca_tot, cb_tot, feats_b)):
                    for c0 in range(0, cn, GMAX):
                        cw = min(GMAX, cn - c0)
                        nc.gpsimd.dma_gather(
                            g[:, base + c0:base + c0 + cw, :], src_ap,
                            isrc[:, ib + (base + c0) * 8:
                                 ib + (base + c0 + cw) * 8],
                            cw * P, cw * P, D, single_packet=False)

                for t in tlist:
                    rows, ca, cb = plan.tiles[t]
                    ch = ca + cb
                    mo = plan.meta_off[t]
                    cols = plan.gcols[t]

                    s = spool.tile([P, plan.ch_max, P], BF16, tag="s")
                    nc.sync.dma_start(s[:, 0:ch, :], sdat[:, mo:mo + ch, :])

                    xt_t = xpool.tile([P, 4, P], BF16, tag="x")
                    nc.sync.dma_start(xt_t[:], xt[t])

                    # aggregation: psum_h[dst, feat] += S_c.T @ G_c
                    ph = phpool.tile([P, D], F32, tag="ph")
                    for c in range(ch):
                        nc.tensor.matmul(
                            ph[:], s[:, c, :], g[:, cols[c], :],
                            start=(c == 0), stop=(c == ch - 1))

                    # h = ph * (1/deg): per-partition scale, PSUM->SBUF on
                    # the scalar engine (DVE would stall behind GpSimd).
                    h = hpool.tile([P, D], BF16, tag="h")
                    nc.scalar.activation(h[:], ph[:], AF.Copy,
                                         scale=inv_s[:, t:t + 1])

                    # transpose h -> hT as plain matmuls against identity
                    ht = htpool.tile([P, 4, P], BF16, tag="ht")
                    ptr = ptrpool.tile([P, 4, P], F32, tag="ptr")
                    for f in range(4):
                        nc.tensor.matmul(
                            ptr[:, f, :], h[:, f * P:(f + 1) * P],
                            ident_s[:], start=True, stop=True)
                    nc.scalar.activation(ht[:], ptr[:], AF.Copy)

                    # out = relu(bias + X @ Wself.T + h @ Wneigh.T)
                    po = popool.tile([P, D], F32, tag="po")
                    nc.tensor.matmul(po[:], ones_s[:], bias_s[:],
                                     start=True, stop=False)
                    for f in range(4):
                        nc.tensor.matmul(po[:], xt_t[:, f, :],
                                         wts_s[:, f, :],
                                         start=False, stop=False)
                        nc.tensor.matmul(po[:], ht[:, f, :],
                                         wtn_s[:, f, :],
                                         start=False, stop=(f == 3))

                    o = opool.tile([P, D], F32, tag="o")
                    nc.scalar.activation(o[:], po[:], AF.Relu)
                    nc.sync.dma_start(out[t * P:t * P + rows, :],
                                      o[:rows, :])

    nc.compile()
    return nc


_cache = {}


def _get_nc(plan):
    k = plan.key()
    if k not in _cache:
        _cache[k] = build(plan)
    return _cache[k]


def kernel(local_feats, src, dst, layer=None, W_self=None, W_neigh=None,
           b=None, **_unused):
    plan, in_maps = _prepare(local_feats, src, dst, W_self, W_neigh, b)
    nc = _get_nc(plan)
    res = run_bass_kernel_spmd(nc, in_maps, core_ids=list(range(NCORES)))
    return np.concatenate([res.results[c]["out"] for c in range(NCORES)],
                          axis=0)
